# revision 23
# baseline (speedup 1.0000x reference)
"""Trainium2 Bass kernel for a BiQRNN3D layer.

reference math:
  gates = conv3d(x, W, SAME, 3x3x3) + b          x: [2,16,31,256,256] f32
  Z, F1, F2 = split(gates, 3, channel)           W: [48,16,3,3,3], b: [48]
  Z = tanh(Z); F1 = sigmoid(F1); F2 = sigmoid(F2)
  h_fwd: depth-forward  recurrence h = F1*h + (1-F1)*Z
  h_bwd: depth-backward recurrence h = F2*h + (1-F2)*Z
  out = h_fwd + h_bwd                            [2,16,31,256,256] f32

Distribution: H (=256) is sharded 32 rows per core across 8 NeuronCores
(SPMD, identical program; each core's x shard carries its 1-row conv halo
with global-edge zeros baked in by the host).

Per-core pipeline:
  * conv as matmul, K = (kd,ci) = 48 contraction rows. The moving x tile
    holds 3 kd-shifted copies in partitions 0-47 (block A) and an
    additional h+1-shifted copy in partitions 64-111 (block B). Partition
    48 is a ones-row (bias rides as a stationary row); 49-63 are zeros.
    The whole tile arrives as TWO large DMAs from a host-prepacked layout
    (plus a one-time aux load for rows 48-63).
  * M = 96: stationary columns (j, co) produce BOTH output h rows of an
    h-block at once. Per psum tile [96, 2*256] six K=112 matmuls
    accumulate: passes (p in {0,1}) x (kw in {0,1,2}); pass p streams x
    rows at tile-h 2p, and blocks A/B provide taps kh = 2p-j and 2p+1-j.
  * F1/F2 stationary columns (weights AND bias) are pre-scaled by 0.5 on
    the host, so ONE Tanh activation per psum tile both evacuates PSUM and
    applies all three nonlinearities: z = tanh(az), t = tanh(af/2) with
    sigmoid(af) = (t+1)/2. Evac writes a per-block SBUF tile ev
    [96, D, 256] fp16; ONE SWDGE DMA per h-block j-row spills it to DRAM
    gates [48, D, S]; XBAR DMA-transpose returns 128-pixel chunks as
    [128, (co,d)].
  * DVE: f = (t+1)/2, zh = z/2, g = (t-1)*zh, tensor_tensor_scan
    (h = f*h - g) forward, and backward via fully-reversed APs (h_bwd
    lands in natural order); f zeroed at the first step of each run so
    one long scan chains safely across channel runs; o = h_fwd + h_bwd
    (fp16) into a per-block batch tile, stored with ONE DMA per h-block.
    Host upcasts / reassembles.
"""

from contextlib import ExitStack

import numpy as np

import concourse.bass as bass
import concourse.tile as tile
from concourse import bacc, mybir

F32 = mybir.dt.float32
F16 = mybir.dt.float16
AF = mybir.ActivationFunctionType
ALU = mybir.AluOpType

N_CORES = 8
B = 2
CIN = 16
HID = 16
CO = 3 * HID            # 48
D = 31
H = 256
W = 256
HSH = H // N_CORES      # 32
HB = 2                  # output h rows per conv tile (= M/CO)
DC = 2                  # d slices per psum tile
WP = W + 2
S = B * HSH * W         # 16384
FX = D * 2 * WP         # x tile free extent per partition
CHUNK = 128
NST = 6                 # stationary matrices
NBLK = B * (HSH // HB)  # 32 h-blocks per core
CD = CO * D


def _build_program(reps=1, do_conv=True, do_scan=True, do_evac=True,
                   do_spill=True, fake_tp=False):
    nc = bacc.Bacc("TRN2", target_bir_lowering=False, debug=False)

    xblk = nc.dram_tensor("x", [NBLK, 96, D, 2, WP], F16,
                          kind="ExternalInput").ap()
    wts = nc.dram_tensor("wts", [128, NST * 2 * CO], F16,
                         kind="ExternalInput").ap()
    aux = nc.dram_tensor("aux", [16, FX], F16, kind="ExternalInput").ap()
    # gates tiled per h-block (512 px) so each XBAR transpose source row is
    # 1KB contiguous (m2s reads concat across col-blocks -> full DMA rate)
    gates = nc.dram_tensor("gates", [S // (HB * W), CD, HB * W], F16,
                           kind="Internal").ap()
    out = nc.dram_tensor("out", [S, HID, D], F16, kind="ExternalOutput").ap()

    with tile.TileContext(nc) as tc, ExitStack() as ctx:
        wsb = nc.alloc_sbuf_tensor("wsb", [128, NST * 2 * CO], F16).ap()
        # x tile: A rows hold x at h = h0 + 2t, B rows x at h0 + 1 + 2t
        xbufs = [nc.alloc_sbuf_tensor(f"xb{i}", [112, D, 2, WP], F16).ap()
                 for i in range(3)]

        nc.sync.dma_start(wsb, wts)
        for xb in xbufs:
            nc.sync.dma_start(
                xb[48:64].rearrange("p a b c -> p (a b c)"), aux)

        ev_pool = ctx.enter_context(tc.tile_pool(name="ev", bufs=2))
        ps_pool = ctx.enter_context(tc.tile_pool(name="ps", bufs=8,
                                                 space="PSUM"))
        t_pool = ctx.enter_context(tc.tile_pool(name="tp", bufs=3))
        sc_pool = ctx.enter_context(tc.tile_pool(name="sc", bufs=4))
        ob_pool = ctx.enter_context(tc.tile_pool(name="ob", bufs=2))

        n_hblk = HSH // HB
        n_dc = (D + DC - 1) // DC

        chunk_q = []
        per_blk = (HB * W) // CHUNK  # 4 chunks per h-block

        def scan_block(s0):
            if not do_scan:
                return
            T4 = t_pool.tile([128, per_blk, CD], F16, tag="T")
            if fake_tp:
                nc.sync.dma_start(
                    T4[:].rearrange("p q r -> p (q r)"),
                    gates.rearrange("a r w -> a (r w)")[0:128,
                                                        0:per_blk * CD])
            else:
                nc.sync.dma_start(T4[:], gates[s0 // (HB * W)],
                                  transpose=True)
            ob = ob_pool.tile([128, per_blk, HID * D], F16, tag="ob")
            for kq in range(per_blk):
                scan_chunk(T4, ob, kq)
            dst = out[s0:s0 + per_blk * CHUNK].rearrange(
                "(q p) c d -> p q (c d)", p=CHUNK)
            nc.gpsimd.dma_start(dst, ob[:])

        def scan_chunk(T, ob, kq):
            Tv = T[:, kq].rearrange("p (c d) -> p c d", d=D)
            T1 = Tv[:, HID:2 * HID]
            T2 = Tv[:, 2 * HID:3 * HID]
            zh = sc_pool.tile([128, HID, D], F16, tag="zh")
            f1 = sc_pool.tile([128, HID, D], F16, tag="f1")
            f2 = sc_pool.tile([128, HID, D], F16, tag="f2")
            g1 = sc_pool.tile([128, HID, D], F16, tag="g1")
            g2 = sc_pool.tile([128, HID, D], F16, tag="g2")
            nc.vector.tensor_scalar_mul(zh[:], Tv[:, 0:HID], 0.5)
            nc.vector.tensor_scalar(f1[:], T1, 0.5, 0.5, ALU.mult, ALU.add)
            nc.vector.tensor_scalar(f2[:], T2, 0.5, 0.5, ALU.mult, ALU.add)
            nc.vector.scalar_tensor_tensor(
                g1[:], T1, 1.0, zh[:], ALU.subtract, ALU.mult)
            nc.vector.scalar_tensor_tensor(
                g2[:], T2, 1.0, zh[:], ALU.subtract, ALU.mult)
            nc.vector.memset(f1[:, :, 0:1], 0.0)
            nc.vector.memset(f2[:, :, D - 1:D], 0.0)
            h1 = sc_pool.tile([128, HID, D], F16, tag="h1")
            h2 = sc_pool.tile([128, HID, D], F16, tag="h2")
            nc.vector.tensor_tensor_scan(
                h1[:].rearrange("p c d -> p (c d)"),
                f1[:].rearrange("p c d -> p (c d)"),
                g1[:].rearrange("p c d -> p (c d)"),
                0.0, ALU.mult, ALU.subtract)
            nc.vector.tensor_tensor_scan(
                h2[:].rearrange("p c d -> p (c d)")[:, ::-1],
                f2[:].rearrange("p c d -> p (c d)")[:, ::-1],
                g2[:].rearrange("p c d -> p (c d)")[:, ::-1],
                0.0, ALU.mult, ALU.subtract)
            nc.vector.tensor_add(
                ob[:, kq].rearrange("p (c d) -> p c d", d=D),
                h1[:], h2[:])

        def load_x(blk):
            xb = xbufs[blk % 3]
            nc.gpsimd.dma_start(xb[0:48], xblk[blk % NBLK, 0:48])
            nc.gpsimd.dma_start(xb[64:112], xblk[blk % NBLK, 48:96])

        nblk_tot = reps * NBLK
        for blk in range(min(3, nblk_tot)):
            load_x(blk)
        for blk in range(nblk_tot):
            xb = xbufs[blk % 3]
            s0 = (blk % NBLK) * HB * W
            ev = ev_pool.tile([96, D, W], F16, tag="ev")
            for dc in range(n_dc if do_conv else 0):
                d0 = dc * DC
                dn = min(DC, D - d0)
                ps = ps_pool.tile([2 * CO, DC * W], F32, tag="ps")
                psv = ps[:, 0:dn * W].rearrange("p (d w) -> p d w", w=W)
                k = 0
                for p in range(2):
                    for kw in range(3):
                        nc.tensor.matmul(
                            psv,
                            wsb[0:112, k * 96:(k + 1) * 96],
                            xb[0:112, d0:d0 + dn, p, kw:kw + W],
                            start=(k == 0), stop=(k == NST - 1))
                        k += 1
                if not do_evac:
                    continue
                evv = ev[:, d0:d0 + dn, :].rearrange("p d w -> p (d w)")
                nc.scalar.activation(evv, ps[:, 0:dn * W], AF.Tanh)
            if blk + 3 < nblk_tot:
                load_x(blk + 3)
            if do_spill and do_evac and do_conv:
                blki = s0 // (HB * W)
                for j in range(HB):
                    dst = gates[blki, :, j * W:(j + 1) * W].rearrange(
                        "(c d) w -> c d w", c=CO)
                    nc.gpsimd.dma_start(dst, ev[j * CO:(j + 1) * CO])
            chunk_q.append(s0)
            while len(chunk_q) > 1:
                scan_block(chunk_q.pop(0))
        while chunk_q:
            scan_block(chunk_q.pop(0))

    nc.finalize()
    return nc


def _host_inputs(x, Wc, b):
    """x: [B, CIN, D, H, W] f32 full input. Returns list of 8 in_maps."""
    bf = np.float16
    # 6 stationaries: idx = p*3+kw, each [128, 96] with cols (j*48+co).
    # rows 0-47 (block A, x at tile-h 2p):   tap kh = 2p - j
    # rows 64-111 (block B, x at h+1):       tap kh = 2p + 1 - j
    wt = np.zeros((NST, 128, 2 * CO), np.float32)
    for p in range(2):
        for kw in range(3):
            idx = p * 3 + kw
            for j in range(2):
                c0 = j * CO
                for blk, khv in ((0, 2 * p - j), (64, 2 * p + 1 - j)):
                    if khv < 0 or khv > 2:
                        continue
                    for kd in range(3):
                        p0 = blk + kd * 16
                        wt[idx, p0:p0 + 16, c0:c0 + CO] = \
                            Wc[:, :, kd, khv, kw].T
    wt[0, 48, 0:CO] = b
    wt[0, 48, CO:2 * CO] = b
    # pre-scale F1/F2 columns (weights and bias) by 0.5 so the single
    # Tanh evac yields t with sigmoid(a) = (t+1)/2
    for j in range(2):
        wt[:, :, j * CO + HID:j * CO + 3 * HID] *= 0.5
    wts = wt.transpose(1, 0, 2).reshape(128, NST * 2 * CO).astype(bf)
    auxa = np.zeros((16, FX), np.float32)
    auxa[0, :] = 1.0
    auxa = auxa.astype(bf)

    n_hblk = HSH // HB
    xt = np.ascontiguousarray(x.transpose(1, 2, 0, 3, 4)).astype(bf)
    in_maps = []
    for c in range(N_CORES):
        hs, he = c * HSH, (c + 1) * HSH
        xp = np.zeros((CIN, D + 2, B, HSH + 2, WP), bf)
        lo = max(hs - 1, 0)
        hi = min(he + 1, H)
        xp[:, 1:D + 1, :, (lo - (hs - 1)):(hi - (hs - 1)), 1:W + 1] = \
            xt[:, :, :, lo:hi, :]
        # pack per-h-block moving tiles: [NBLK, 96, D, 2, WP]
        # rows kd*16+ci      = xp[ci, kd+d, b, 2*hb + 2t, w]   (block A)
        # rows 48+kd*16+ci   = xp[ci, kd+d, b, 2*hb + 2t + 1, w] (block B)
        xbk = np.empty((B, n_hblk, 96, D, 2, WP), bf)
        for kd in range(3):
            sl = xp[:, kd:kd + D]            # [CIN, D, B, HSH+2, WP]
            for t in range(2):
                ha = np.arange(n_hblk) * HB + 2 * t
                arr = sl[:, :, :, ha, :].transpose(2, 3, 0, 1, 4)
                xbk[:, :, kd * 16:kd * 16 + 16, :, t, :] = arr
                arr = sl[:, :, :, ha + 1, :].transpose(2, 3, 0, 1, 4)
                xbk[:, :, 48 + kd * 16:48 + kd * 16 + 16, :, t, :] = arr
        xbk = xbk.reshape(NBLK, 96, D, 2, WP)
        in_maps.append({"x": xbk, "wts": wts, "aux": auxa})
    return in_maps


_PROGRAM = None


def _get_program():
    global _PROGRAM
    if _PROGRAM is None:
        _PROGRAM = _build_program()
    return _PROGRAM


def run_sharded(in_maps, trace=False, **kw):
    from concourse import bass_utils
    nc = _get_program()
    return bass_utils.run_bass_kernel_spmd(
        nc, in_maps, core_ids=list(range(N_CORES)), trace=trace, **kw)


def _assemble(results):
    outf = np.empty((B, HID, D, H, W), np.float32)
    for c in range(N_CORES):
        raw = np.asarray(results[c]["out"]).astype(np.float32)
        o = raw.reshape(B, HSH, W, HID, D).transpose(0, 3, 4, 1, 2)
        outf[:, :, :, c * HSH:(c + 1) * HSH, :] = o
    return outf


def kernel(x, W, b):
    x = np.asarray(x, np.float32)
    W = np.asarray(W, np.float32)
    b = np.asarray(b, np.float32)
    in_maps = _host_inputs(x, W, b)
    res = run_sharded(in_maps)
    return _assemble(res.results)


# revision 30
# speedup vs baseline: 1.2410x; 1.2410x over previous
"""Trainium2 Bass kernel for a BiQRNN3D layer.

reference math:
  gates = conv3d(x, W, SAME, 3x3x3) + b          x: [2,16,31,256,256] f32
  Z, F1, F2 = split(gates, 3, channel)           W: [48,16,3,3,3], b: [48]
  Z = tanh(Z); F1 = sigmoid(F1); F2 = sigmoid(F2)
  h_fwd: depth-forward  recurrence h = F1*h + (1-F1)*Z
  h_bwd: depth-backward recurrence h = F2*h + (1-F2)*Z
  out = h_fwd + h_bwd                            [2,16,31,256,256] f32

Distribution: H (=256) is sharded 32 rows per core across 8 NeuronCores
(SPMD, identical program; each core's x shard carries its 1-row conv halo
with global-edge zeros baked in by the host).

Per-core pipeline:
  * conv as matmul, K = (kd,ci) = 48 contraction rows. The moving x tile
    holds 3 kd-shifted copies in partitions 0-47 (block A) and an
    additional h+1-shifted copy in partitions 64-111 (block B). Partition
    48 is a ones-row (bias rides as a stationary row); 49-63 are zeros.
    The whole tile arrives as TWO large DMAs from a host-prepacked layout
    (plus a one-time aux load for rows 48-63).
  * M = 96: stationary columns (j, co) produce BOTH output h rows of an
    h-block at once. Per psum tile [96, 2*256] six K=112 matmuls
    accumulate: passes (p in {0,1}) x (kw in {0,1,2}); pass p streams x
    rows at tile-h 2p, and blocks A/B provide taps kh = 2p-j and 2p+1-j.
  * F1/F2 stationary columns (weights AND bias) are pre-scaled by 0.5 on
    the host, so ONE Tanh activation per psum tile both evacuates PSUM and
    applies all three nonlinearities: z = tanh(az), t = tanh(af/2) with
    sigmoid(af) = (t+1)/2. Evac writes a per-block SBUF tile ev
    [96, D, 256] fp16; ONE SWDGE DMA per h-block j-row spills it to DRAM
    gates [48, D, S]; XBAR DMA-transpose returns 128-pixel chunks as
    [128, (co,d)].
  * DVE: f = (t+1)/2, zh = z/2, g = (t-1)*zh, tensor_tensor_scan
    (h = f*h - g) forward, and backward via fully-reversed APs (h_bwd
    lands in natural order); f zeroed at the first step of each run so
    one long scan chains safely across channel runs; o = h_fwd + h_bwd
    (fp16) into a per-block batch tile, stored with ONE DMA per h-block.
    Host upcasts / reassembles.
"""

from contextlib import ExitStack

import numpy as np

import concourse.bass as bass
import concourse.tile as tile
from concourse import bacc, mybir

F32 = mybir.dt.float32
F16 = mybir.dt.float16
AF = mybir.ActivationFunctionType
ALU = mybir.AluOpType

N_CORES = 8
B = 2
CIN = 16
HID = 16
CO = 3 * HID            # 48
D = 31
H = 256
W = 256
HSH = H // N_CORES      # 32
HB = 2                  # output h rows per conv tile (= M/CO)
DC = 2                  # d slices per psum tile
WP = W + 2
S = B * HSH * W         # 16384
FX = D * 2 * WP         # x tile free extent per partition
CHUNK = 128
NST = 6                 # stationary matrices
NBLK = B * (HSH // HB)  # 32 h-blocks per core
CD = CO * D


def _build_program(reps=1, do_conv=True, do_scan=True, do_evac=True,
                   do_spill=True, fake_tp=False):
    nc = bacc.Bacc("TRN2", target_bir_lowering=False, debug=False)

    xblk = nc.dram_tensor("x", [NBLK, 96, D, 2, WP], F16,
                          kind="ExternalInput").ap()
    wts = nc.dram_tensor("wts", [128, NST * 2 * CO], F16,
                         kind="ExternalInput").ap()
    aux = nc.dram_tensor("aux", [16, FX], F16, kind="ExternalInput").ap()
    # gates tiled per 128-px chunk: transpose sources are contiguous and
    # spill writes coalesce into 4KB packets
    gates = nc.dram_tensor("gates", [S // CHUNK, CD, CHUNK], F16,
                           kind="Internal").ap()
    out = nc.dram_tensor("out", [S, HID, D], F16, kind="ExternalOutput").ap()

    with tile.TileContext(nc) as tc, ExitStack() as ctx:
        wsb = nc.alloc_sbuf_tensor("wsb", [128, NST * 2 * CO], F16).ap()
        # x tile: A rows hold x at h = h0 + 2t, B rows x at h0 + 1 + 2t
        xbufs = [nc.alloc_sbuf_tensor(f"xb{i}", [112, D, 2, WP], F16).ap()
                 for i in range(3)]

        nc.sync.dma_start(wsb, wts)
        for xb in xbufs:
            nc.sync.dma_start(
                xb[48:64].rearrange("p a b c -> p (a b c)"), aux)

        ev_pool = ctx.enter_context(tc.tile_pool(name="ev", bufs=2))
        ps_pool = ctx.enter_context(tc.tile_pool(name="ps", bufs=8,
                                                 space="PSUM"))
        t_pool = ctx.enter_context(tc.tile_pool(name="tp", bufs=8))
        sc_pool = ctx.enter_context(tc.tile_pool(name="sc", bufs=4))
        ob_pool = ctx.enter_context(tc.tile_pool(name="ob", bufs=2))

        n_hblk = HSH // HB
        n_dc = (D + DC - 1) // DC

        chunk_q = []
        per_blk = (HB * W) // CHUNK  # 4 chunks per h-block

        def scan_block(s0):
            if not do_scan:
                return
            Ts = []
            for kq in range(per_blk):
                ch = s0 // CHUNK + kq
                T = t_pool.tile([128, CD], F16, tag="T")
                if fake_tp:
                    nc.sync.dma_start(
                        T[:],
                        gates.rearrange("a r w -> a (r w)")[0:128, 0:CD])
                else:
                    nc.sync.dma_start(T[:], gates[ch], transpose=True)
                Ts.append(T)
            ob = ob_pool.tile([128, per_blk, HID * D], F16, tag="ob")
            for kq, T in enumerate(Ts):
                scan_chunk(T, ob, kq)
            dst = out[s0:s0 + per_blk * CHUNK].rearrange(
                "(q p) c d -> p q (c d)", p=CHUNK)
            nc.gpsimd.dma_start(dst, ob[:])

        def scan_chunk(T, ob, kq):
            # g' = (t-1)*z is 2x the true g; the recurrence is linear in g,
            # so h' = 2h and the host halves the output.
            Tv = T[:].rearrange("p (c d) -> p c d", d=D)
            Tz = Tv[:, 0:HID]
            T1 = Tv[:, HID:2 * HID]
            T2 = Tv[:, 2 * HID:3 * HID]
            f1 = sc_pool.tile([128, HID, D], F16, tag="f1")
            f2 = sc_pool.tile([128, HID, D], F16, tag="f2")
            g1 = sc_pool.tile([128, HID, D], F16, tag="g1")
            g2 = sc_pool.tile([128, HID, D], F16, tag="g2")
            nc.vector.tensor_scalar(f1[:], T1, 0.5, 0.5, ALU.mult, ALU.add)
            nc.vector.tensor_scalar(f2[:], T2, 0.5, 0.5, ALU.mult, ALU.add)
            nc.vector.scalar_tensor_tensor(
                g1[:], T1, 1.0, Tz, ALU.subtract, ALU.mult)
            nc.vector.scalar_tensor_tensor(
                g2[:], T2, 1.0, Tz, ALU.subtract, ALU.mult)
            nc.vector.memset(f1[:, :, 0:1], 0.0)
            nc.vector.memset(f2[:, :, D - 1:D], 0.0)
            h1 = sc_pool.tile([128, HID, D], F16, tag="h1")
            h2 = sc_pool.tile([128, HID, D], F16, tag="h2")
            nc.vector.tensor_tensor_scan(
                h1[:].rearrange("p c d -> p (c d)"),
                f1[:].rearrange("p c d -> p (c d)"),
                g1[:].rearrange("p c d -> p (c d)"),
                0.0, ALU.mult, ALU.subtract)
            nc.vector.tensor_tensor_scan(
                h2[:].rearrange("p c d -> p (c d)")[:, ::-1],
                f2[:].rearrange("p c d -> p (c d)")[:, ::-1],
                g2[:].rearrange("p c d -> p (c d)")[:, ::-1],
                0.0, ALU.mult, ALU.subtract)
            nc.vector.tensor_add(
                ob[:, kq].rearrange("p (c d) -> p c d", d=D),
                h1[:], h2[:])

        def load_x(blk):
            xb = xbufs[blk % 3]
            nc.sync.dma_start(xb[0:48], xblk[blk % NBLK, 0:48])
            nc.sync.dma_start(xb[64:112], xblk[blk % NBLK, 48:96])

        nblk_tot = reps * NBLK
        for blk in range(min(3, nblk_tot)):
            load_x(blk)
        for blk in range(nblk_tot):
            xb = xbufs[blk % 3]
            s0 = (blk % NBLK) * HB * W
            ev = ev_pool.tile([96, D, W], F16, tag="ev")
            for dc in range(n_dc if do_conv else 0):
                d0 = dc * DC
                dn = min(DC, D - d0)
                ps = ps_pool.tile([2 * CO, DC * W], F32, tag="ps")
                psv = ps[:, 0:dn * W].rearrange("p (d w) -> p d w", w=W)
                k = 0
                for p in range(2):
                    for kw in range(3):
                        nc.tensor.matmul(
                            psv,
                            wsb[0:112, k * 96:(k + 1) * 96],
                            xb[0:112, d0:d0 + dn, p, kw:kw + W],
                            start=(k == 0), stop=(k == NST - 1))
                        k += 1
                if not do_evac:
                    continue
                evv = ev[:, d0:d0 + dn, :].rearrange("p d w -> p (d w)")
                nc.scalar.activation(evv, ps[:, 0:dn * W], AF.Tanh)
            if do_spill and do_evac and do_conv:
                ch0 = s0 // CHUNK
                for j in range(HB):
                    dst = gates[ch0 + 2 * j:ch0 + 2 * j + 2].rearrange(
                        "b (c d) w -> c d b w", c=CO)
                    src = ev[j * CO:(j + 1) * CO].rearrange(
                        "p d (b w) -> p d b w", b=2)
                    nc.gpsimd.dma_start(dst, src)
            chunk_q.append(s0)
            while len(chunk_q) > 1:
                scan_block(chunk_q.pop(0))
            if blk + 3 < nblk_tot:
                load_x(blk + 3)
        while chunk_q:
            scan_block(chunk_q.pop(0))

    nc.finalize()
    return nc


def _host_inputs(x, Wc, b):
    """x: [B, CIN, D, H, W] f32 full input. Returns list of 8 in_maps."""
    bf = np.float16
    # 6 stationaries: idx = p*3+kw, each [128, 96] with cols (j*48+co).
    # rows 0-47 (block A, x at tile-h 2p):   tap kh = 2p - j
    # rows 64-111 (block B, x at h+1):       tap kh = 2p + 1 - j
    wt = np.zeros((NST, 128, 2 * CO), np.float32)
    for p in range(2):
        for kw in range(3):
            idx = p * 3 + kw
            for j in range(2):
                c0 = j * CO
                for blk, khv in ((0, 2 * p - j), (64, 2 * p + 1 - j)):
                    if khv < 0 or khv > 2:
                        continue
                    for kd in range(3):
                        p0 = blk + kd * 16
                        wt[idx, p0:p0 + 16, c0:c0 + CO] = \
                            Wc[:, :, kd, khv, kw].T
    wt[0, 48, 0:CO] = b
    wt[0, 48, CO:2 * CO] = b
    # pre-scale F1/F2 columns (weights and bias) by 0.5 so the single
    # Tanh evac yields t with sigmoid(a) = (t+1)/2
    for j in range(2):
        wt[:, :, j * CO + HID:j * CO + 3 * HID] *= 0.5
    wts = wt.transpose(1, 0, 2).reshape(128, NST * 2 * CO).astype(bf)
    auxa = np.zeros((16, FX), np.float32)
    auxa[0, :] = 1.0
    auxa = auxa.astype(bf)

    n_hblk = HSH // HB
    xt = np.ascontiguousarray(x.transpose(1, 2, 0, 3, 4)).astype(bf)
    in_maps = []
    for c in range(N_CORES):
        hs, he = c * HSH, (c + 1) * HSH
        xp = np.zeros((CIN, D + 2, B, HSH + 2, WP), bf)
        lo = max(hs - 1, 0)
        hi = min(he + 1, H)
        xp[:, 1:D + 1, :, (lo - (hs - 1)):(hi - (hs - 1)), 1:W + 1] = \
            xt[:, :, :, lo:hi, :]
        # pack per-h-block moving tiles: [NBLK, 96, D, 2, WP]
        # rows kd*16+ci      = xp[ci, kd+d, b, 2*hb + 2t, w]   (block A)
        # rows 48+kd*16+ci   = xp[ci, kd+d, b, 2*hb + 2t + 1, w] (block B)
        xbk = np.empty((B, n_hblk, 96, D, 2, WP), bf)
        for kd in range(3):
            sl = xp[:, kd:kd + D]            # [CIN, D, B, HSH+2, WP]
            for t in range(2):
                ha = np.arange(n_hblk) * HB + 2 * t
                arr = sl[:, :, :, ha, :].transpose(2, 3, 0, 1, 4)
                xbk[:, :, kd * 16:kd * 16 + 16, :, t, :] = arr
                arr = sl[:, :, :, ha + 1, :].transpose(2, 3, 0, 1, 4)
                xbk[:, :, 48 + kd * 16:48 + kd * 16 + 16, :, t, :] = arr
        xbk = xbk.reshape(NBLK, 96, D, 2, WP)
        in_maps.append({"x": xbk, "wts": wts, "aux": auxa})
    return in_maps


_PROGRAM = None


def _get_program():
    global _PROGRAM
    if _PROGRAM is None:
        _PROGRAM = _build_program()
    return _PROGRAM


def run_sharded(in_maps, trace=False, **kw):
    from concourse import bass_utils
    nc = _get_program()
    return bass_utils.run_bass_kernel_spmd(
        nc, in_maps, core_ids=list(range(N_CORES)), trace=trace, **kw)


def _assemble(results):
    outf = np.empty((B, HID, D, H, W), np.float32)
    for c in range(N_CORES):
        raw = np.asarray(results[c]["out"]).astype(np.float32) * 0.5
        o = raw.reshape(B, HSH, W, HID, D).transpose(0, 3, 4, 1, 2)
        outf[:, :, :, c * HSH:(c + 1) * HSH, :] = o
    return outf


def kernel(x, W, b):
    x = np.asarray(x, np.float32)
    W = np.asarray(W, np.float32)
    b = np.asarray(b, np.float32)
    in_maps = _host_inputs(x, W, b)
    res = run_sharded(in_maps)
    return _assemble(res.results)
